# revision 1
# baseline (speedup 1.0000x reference)
"""CAM (channel-attention) + SE module kernel for TRN2, batch-parallel over 8 cores.

Per sample (C=256, N=9216):
  v = x.reshape(C, N)
  E = v @ v.T                      (energy; fp32r matmuls on PE)
  att = softmax(-E, axis=-1)       (rows; stabilized at row-min of E)
  pooled = mean(x) over N          (free reduction riding the x-load cast)
  gate = sigmoid(w2 @ relu(w1 @ pooled + b1) + b2)
  out = gamma * gate[:,None] * (att @ v) + x

v2 layout: x lives on-chip as fp32r (rounded once during load, on ACT with
accum_out giving pooled for free). Energy transposes run in fp32r (1.5cyc/row),
phase-2 rhs needs no casts, and the residual uses the rounded x (1e-5 rel).
"""
import numpy as np
import concourse.bass as bass
import concourse.bacc as bacc
import concourse.tile as tile
import concourse.mybir as mybir
from concourse.bass_utils import run_bass_kernel_spmd
import concourse.bass_utils as _bu

# Re-enable walrus LDWEIGHTS optimization (elides/backgrounds redundant weight
# loads). bass_utils hardcodes --enable-ldw-opt=false; flip it on our compiles.
if not getattr(_bu.run_command, "_ldw_patched", False):
    _orig_run_command = _bu.run_command

    def _run_command_ldw(argv, **kwargs):
        argv = ["--enable-ldw-opt=true" if a == "--enable-ldw-opt=false" else a
                for a in argv]
        return _orig_run_command(argv, **kwargs)

    _run_command_ldw._ldw_patched = True
    _bu.run_command = _run_command_ldw

F32 = mybir.dt.float32
F32R = mybir.dt.float32r

B, C, H, W = 16, 256, 96, 96
N = H * W                 # 9216
NCORES = 8
BL = B // NCORES          # samples per core
NCH = N // 128            # 72 n-chunks for the energy phase
NT = 512                  # phase-2 psum tile (one PSUM bank of fp32)
SEG = 1536                # x-load segment columns
NSEG = N // SEG           # 6
OUTCH = 2048              # output DMA chunk (columns)
R = C // 8                # 32 (SE hidden dim)


def build_nc():
    nc = bacc.Bacc("TRN2", target_bir_lowering=False, debug=False, num_devices=NCORES)

    x_d = nc.dram_tensor("x", [BL, C, N], F32, kind="ExternalInput")
    gamma_d = nc.dram_tensor("gamma", [1], F32, kind="ExternalInput")
    w1_d = nc.dram_tensor("w1", [R, C], F32, kind="ExternalInput")   # pre-scaled by 1/N
    b1_d = nc.dram_tensor("b1", [R], F32, kind="ExternalInput")
    w2_d = nc.dram_tensor("w2", [C, R], F32, kind="ExternalInput")
    b2_d = nc.dram_tensor("b2", [C], F32, kind="ExternalInput")
    ident_d = nc.dram_tensor("ident", [128, 128], F32, kind="ExternalInput")
    out_d = nc.dram_tensor("out", [BL, C, N], F32, kind="ExternalOutput")

    with tile.TileContext(nc) as tc:
        with (
            tc.tile_pool(name="px", bufs=2 * BL) as px,
            tc.tile_pool(name="pstage", bufs=3) as pstage,
            tc.tile_pool(name="pxT", bufs=4) as pxT,
            tc.tile_pool(name="patt", bufs=2) as patt,
            tc.tile_pool(name="pout", bufs=2) as pout,
            tc.tile_pool(name="psmall", bufs=2) as psmall,
            tc.tile_pool(name="psingle", bufs=1) as psingle,
            tc.tile_pool(name="ppsE", bufs=1, space="PSUM") as ppsE,
            tc.tile_pool(name="ppsX", bufs=3, space="PSUM") as ppsX,
            tc.tile_pool(name="ppsO", bufs=2, space="PSUM") as ppsO,
        ):
            # ---------------- parameter prep (once) ----------------
            ident = psingle.tile([128, 128], F32, name="ident")
            nc.gpsimd.dma_start(out=ident[:], in_=ident_d[:])
            identr = psingle.tile([128, 128], F32R, name="identr")
            nc.vector.tensor_copy(out=identr[:], in_=ident[:])
            gamma_sb = psingle.tile([128, 1], F32, name="gamma_sb")
            nc.gpsimd.dma_start(
                out=gamma_sb[:],
                in_=bass.AP(tensor=gamma_d.ap().tensor, offset=0, ap=[[0, 128], [1, 1]]),
            )
            b1_sb = psingle.tile([R, 1], F32, name="b1_sb")
            nc.gpsimd.dma_start(
                out=b1_sb[:],
                in_=bass.AP(tensor=b1_d.ap().tensor, offset=0, ap=[[1, R], [1, 1]]),
            )
            b2_sb = psingle.tile([128, 2], F32, name="b2_sb")
            nc.gpsimd.dma_start(out=b2_sb[:], in_=b2_d[:].rearrange("(h c) -> c h", c=128))

            # w1T[c, h, r] = w1[r, h*128+c]
            w1_nat = psingle.tile([R, 2, 128], F32, name="w1_nat")
            nc.gpsimd.dma_start(out=w1_nat[:], in_=w1_d[:].rearrange("r (h c) -> r h c", c=128))
            w1T_ps = ppsX.tile([128, 2, R], F32, tag="psx", name="w1T_ps")
            for h in range(2):
                nc.tensor.transpose(w1T_ps[:, h, :], w1_nat[:, h, :], ident[0:R, 0:R])
            w1T = psingle.tile([128, 2, R], F32, name="w1T")
            nc.vector.tensor_copy(out=w1T[:], in_=w1T_ps[:])

            # w2T[r, h*128+c] = w2[h*128+c, r]
            w2_nat = psingle.tile([128, 2, R], F32, name="w2_nat")
            nc.gpsimd.dma_start(out=w2_nat[:], in_=w2_d[:].rearrange("(h c) r -> c h r", c=128))
            w2T = psingle.tile([R, 2, 128], F32, name="w2T")
            for h in range(2):
                w2T_ps = ppsX.tile([R, 128], F32, tag="psx", name=f"w2T_ps_{h}")
                nc.tensor.transpose(w2T_ps[:], w2_nat[:, h, :], ident[:])
                nc.vector.tensor_copy(out=w2T[:, h, :], in_=w2T_ps[:])

            # ---------------- per sample (software-pipelined) ----------------
            x_sb = {}
            pp = {}
            psE = {}
            att = {}
            attT = {}

            def emit_load(b):
                pp[b] = psmall.tile([128, 2, NSEG], F32, tag="pp", name=f"pp_{b}")
                x_sb[b] = [
                    px.tile([128, N], F32R, tag="xsb", name=f"x_{b}_{h}")
                    for h in range(2)
                ]
                for g in range(NSEG):
                    sl = slice(SEG * g, SEG * (g + 1))
                    for h in range(2):
                        st = pstage.tile([128, SEG], F32, tag="stage", name=f"st_{b}_{h}_{g}")
                        nc.gpsimd.dma_start(
                            out=st[:], in_=x_d[b, 128 * h:128 * (h + 1), sl],
                        )
                        nc.scalar.activation(
                            out=x_sb[b][h][:, sl], in_=st[:],
                            func=mybir.ActivationFunctionType.Copy,
                            accum_out=pp[b][:, h, g:g + 1],
                        )

            def emit_phase1(b):
                psE[b] = ppsE.tile([128, 512], F32, tag="psE", name=f"psE_{b}")
                GRP = 3
                for g0 in range(0, NCH // 2, GRP):
                    k2s = range(g0, min(g0 + GRP, NCH // 2))
                    xTs = {}
                    for k2 in k2s:
                        xT_ps = ppsX.tile([128, 512], F32R, tag="psx", name=f"xTps_{b}_{k2}")
                        for sub in range(2):
                            k = 2 * k2 + sub
                            for h in range(2):
                                nc.tensor.transpose(
                                    xT_ps[:, 256 * sub + 128 * h:256 * sub + 128 * (h + 1)],
                                    x_sb[b][h][:, 128 * k:128 * (k + 1)],
                                    identr[:],
                                )
                        xT = pxT.tile([128, 512], F32R, tag="xT", name=f"xT_{b}_{k2}")
                        if k2 % 3 == 1:
                            nc.scalar.copy(out=xT[:], in_=xT_ps[:])
                        else:
                            nc.vector.tensor_copy(out=xT[:], in_=xT_ps[:])
                        xTs[k2] = xT
                    for k2 in k2s:
                        for sub in range(2):
                            for h in range(2):
                                nc.tensor.matmul(
                                    psE[b][:, 256 * h:256 * (h + 1)],
                                    xTs[k2][:, 256 * sub + 128 * h:256 * sub + 128 * (h + 1)],
                                    xTs[k2][:, 256 * sub:256 * (sub + 1)],
                                    start=(k2 == 0 and sub == 0 and h == 0),
                                    stop=(k2 == NCH // 2 - 1 and sub == 1 and h == 1),
                                )

            def emit_epilogue(b):
                # SE gate (w1 pre-scaled by 1/N on host)
                pooled = psmall.tile([128, 2], F32, tag="pooled", name=f"pooled_{b}")
                for h in range(2):
                    nc.vector.reduce_sum(
                        out=pooled[:, h:h + 1], in_=pp[b][:, h, :], axis=mybir.AxisListType.X,
                    )
                hid_ps = ppsX.tile([R, 1], F32, tag="psx", name=f"hid_ps_{b}")
                for h in range(2):
                    nc.tensor.matmul(
                        hid_ps[:], w1T[:, h, :], pooled[:, h:h + 1],
                        start=(h == 0), stop=(h == 1),
                    )
                hid = psmall.tile([R, 1], F32, tag="hid", name=f"hid_{b}")
                nc.scalar.activation(
                    out=hid[:], in_=hid_ps[:],
                    func=mybir.ActivationFunctionType.Relu, bias=b1_sb[:], scale=1.0,
                )
                gg = psmall.tile([128, 2], F32, tag="gg", name=f"gg_{b}")
                for h in range(2):
                    gate_ps = ppsX.tile([128, 1], F32, tag="psx", name=f"gate_ps_{b}_{h}")
                    nc.tensor.matmul(gate_ps[:], w2T[:, h, :], hid[:])
                    nc.scalar.activation(
                        out=gg[:, h:h + 1], in_=gate_ps[:],
                        func=mybir.ActivationFunctionType.Sigmoid,
                        bias=b2_sb[:, h:h + 1], scale=1.0,
                    )
                nc.vector.tensor_scalar_mul(out=gg[:], in0=gg[:], scalar1=gamma_sb[:])

                # softmax rows + fold in gamma*gate
                att[b] = []
                for h in range(2):
                    pE = psE[b][:, 256 * h:256 * (h + 1)]
                    mn = psmall.tile([128, 1], F32, tag="mn", name=f"mn_{b}_{h}")
                    nc.vector.tensor_reduce(
                        out=mn[:], in_=pE,
                        axis=mybir.AxisListType.X, op=mybir.AluOpType.min,
                    )
                    s = psmall.tile([128, 1], F32, tag="s", name=f"s_{b}_{h}")
                    at = patt.tile([128, 256], F32, tag=f"att{h}", bufs=1, name=f"att_{b}_{h}")
                    nc.scalar.activation(
                        out=at[:], in_=pE,
                        func=mybir.ActivationFunctionType.Exp,
                        bias=mn[:], scale=-1.0, accum_out=s[:],
                    )
                    rs = psmall.tile([128, 1], F32, tag="rs", name=f"rs_{b}_{h}")
                    nc.vector.reciprocal(out=rs[:], in_=s[:])
                    nc.vector.tensor_mul(out=rs[:], in0=rs[:], in1=gg[:, h:h + 1])
                    nc.vector.tensor_scalar_mul(out=at[:], in0=at[:], scalar1=rs[:])
                    att[b].append(at)

                # transpose attention (f32 PE transpose, round on ACT copy)
                attT[b] = patt.tile([128, 2, 256], F32R, tag="attT", name=f"attT_{b}")
                for j in range(2):
                    attT_ps = ppsX.tile([128, 256], F32, tag="psx", name=f"attTps_{b}_{j}")
                    for h in range(2):
                        nc.tensor.transpose(
                            attT_ps[:, 128 * h:128 * (h + 1)],
                            att[b][h][:, 128 * j:128 * (j + 1)],
                            ident[:],
                        )
                    nc.scalar.copy(out=attT[b][:, j, :], in_=attT_ps[:])

            def emit_phase2(b):
                ncols = [OUTCH] * (N // OUTCH) + ([N % OUTCH] if N % OUTCH else [])
                for h in range(2):
                    col0 = 0
                    for ci, cw in enumerate(ncols):
                        o_sb = pout.tile([128, OUTCH], F32, tag="osb", name=f"o_{b}_{ci}_{h}")
                        ngr = cw // 1024
                        pso_g = []
                        for gg2 in range(ngr):
                            pso_g.append(ppsO.tile([128, 1024], F32, tag="ps_o", name=f"pso_{b}_{ci}_{gg2}_{h}"))
                        for j in range(2):
                            for gg2 in range(ngr):
                                for tt in range(2):
                                    n0 = col0 + gg2 * 1024 + tt * NT
                                    nc.tensor.matmul(
                                        pso_g[gg2][:, tt * NT:(tt + 1) * NT],
                                        attT[b][:, j, 128 * h:128 * (h + 1)],
                                        x_sb[b][j][:, n0:n0 + NT],
                                        start=(j == 0), stop=(j == 1),
                                    )
                        for gg2 in range(ngr):
                            g0 = col0 + gg2 * 1024
                            nc.vector.tensor_add(
                                out=o_sb[:, gg2 * 1024:(gg2 + 1) * 1024],
                                in0=pso_g[gg2][:],
                                in1=x_sb[b][h][:, g0:g0 + 1024],
                            )
                        nc.gpsimd.dma_start(
                            out=out_d[b, 128 * h:128 * (h + 1), col0:col0 + cw],
                            in_=o_sb[:, 0:cw],
                        )
                        col0 += cw

            emit_load(0)
            emit_load(1)
            emit_phase1(0)
            emit_epilogue(0)
            emit_phase1(1)
            emit_phase2(0)
            emit_epilogue(1)
            emit_phase2(1)

    nc.finalize()
    return nc


_CACHE = {}


def get_nc():
    if "nc" not in _CACHE:
        _CACHE["nc"] = build_nc()
    return _CACHE["nc"]


def kernel_with_result(x, gamma, w1, b1, w2, b2, trace=False, **_ignored):
    x = np.asarray(x, dtype=np.float32)
    nc = get_nc()
    params = {
        "gamma": np.asarray(gamma, np.float32).reshape(1),
        "w1": np.asarray(w1, np.float32) * np.float32(1.0 / N),
        "b1": np.asarray(b1, np.float32),
        "w2": np.asarray(w2, np.float32),
        "b2": np.asarray(b2, np.float32),
        "ident": np.eye(128, dtype=np.float32),
    }
    xr = x.reshape(B, C, N)
    in_maps = [dict(params, x=xr[i * BL:(i + 1) * BL]) for i in range(NCORES)]
    res = run_bass_kernel_spmd(nc, in_maps, core_ids=list(range(NCORES)), trace=trace)
    out = np.concatenate([res.results[i]["out"] for i in range(NCORES)], axis=0)
    return out.reshape(B, C, H, W), res


def kernel(x, gamma, w1, b1, w2, b2, **_ignored):
    out, _res = kernel_with_result(x, gamma, w1, b1, w2, b2, trace=False)
    return out



# revision 6
# speedup vs baseline: 1.1894x; 1.1894x over previous
"""CAM (channel-attention) + SE module kernel for TRN2, batch-parallel over 8 cores.

Per sample (C=256, N=9216):
  v = x.reshape(C, N)
  E = v @ v.T                      (energy; fp16 matmuls, fp32 PSUM accum)
  att = exp(rowmin(E) - E)         (unnormalized, fp16, straight from ACT)
  pooled = mean(x) over N          (free reduction riding the x-load cast)
  gate = sigmoid(w2 @ relu(w1 @ pooled + b1) + b2)
  scale_c = gamma * gate_c / sum_d att[c,d]
  out = scale_c * (att @ v) + x    (fused scalar_tensor_tensor epilogue)

v3 layout: x lives on-chip as fp16 only (rounded during load on ACT, accum_out
giving pooled for free). All PE traffic is fp16: FWL halves LDWEIGHTS,
transposes run 1cyc/row, phase-2 rhs streams 512-col slabs. Softmax
normalization, gamma and the SE gate are folded into one per-partition scalar
applied by the fused scale+residual-add (DVE/GPSIMD scalar_tensor_tensor).
"""
import numpy as np
import concourse.bass as bass
import concourse.bacc as bacc
import concourse.tile as tile
import concourse.mybir as mybir
from concourse.bass_utils import run_bass_kernel_spmd

# NOTE: --enable-ldw-opt=true (walrus LDW elision) is incompatible with the
# standalone InstLdweights that bass emits for 16-bit weights; rely on the
# PE's hardware reorder window to background weight loads instead.

F32 = mybir.dt.float32
F16 = mybir.dt.float16

B, C, H, W = 16, 256, 96, 96
N = H * W                 # 9216
NCORES = 8
BL = B // NCORES          # samples per core
NCH = N // 128            # 72 n-chunks for the energy phase
GRP = 4                   # chunks per phase-1 group (one full PSUM bank of fp16)
NT = 512                  # phase-2 psum tile (one PSUM bank of fp32)
SEG = 1536                # x-load segment columns
NSEG = N // SEG           # 6
OUTCH = 2048              # output DMA chunk (columns)
R = C // 8                # 32 (SE hidden dim)


def build_nc():
    nc = bacc.Bacc("TRN2", target_bir_lowering=False, debug=False, num_devices=NCORES)

    x_d = nc.dram_tensor("x", [BL, C, N], F32, kind="ExternalInput")
    gamma_d = nc.dram_tensor("gamma", [1], F32, kind="ExternalInput")
    w1_d = nc.dram_tensor("w1", [R, C], F32, kind="ExternalInput")   # pre-scaled by 1/N
    b1_d = nc.dram_tensor("b1", [R], F32, kind="ExternalInput")
    w2_d = nc.dram_tensor("w2", [C, R], F32, kind="ExternalInput")
    b2_d = nc.dram_tensor("b2", [C], F32, kind="ExternalInput")
    ident_d = nc.dram_tensor("ident", [128, 128], F32, kind="ExternalInput")
    out_d = nc.dram_tensor("out", [BL, C, N], F32, kind="ExternalOutput")

    with tile.TileContext(nc) as tc:
        with (
            tc.tile_pool(name="px", bufs=2 * BL) as px,
            tc.tile_pool(name="pstage", bufs=3) as pstage,
            tc.tile_pool(name="pxT", bufs=3) as pxT,
            tc.tile_pool(name="patt", bufs=2) as patt,
            tc.tile_pool(name="pout", bufs=3) as pout,
            tc.tile_pool(name="psmall", bufs=2) as psmall,
            tc.tile_pool(name="psingle", bufs=1) as psingle,
            tc.tile_pool(name="ppsE", bufs=1, space="PSUM") as ppsE,
            tc.tile_pool(name="ppsX", bufs=3, space="PSUM") as ppsX,
            tc.tile_pool(name="ppsO", bufs=4, space="PSUM") as ppsO,
        ):
            # ---------------- parameter prep (once) ----------------
            ident = psingle.tile([128, 128], F32, name="ident")
            nc.gpsimd.dma_start(out=ident[:], in_=ident_d[:])
            ident16 = psingle.tile([128, 128], F16, name="ident16")
            nc.vector.tensor_copy(out=ident16[:], in_=ident[:])
            gamma_sb = psingle.tile([128, 1], F32, name="gamma_sb")
            nc.gpsimd.dma_start(
                out=gamma_sb[:],
                in_=bass.AP(tensor=gamma_d.ap().tensor, offset=0, ap=[[0, 128], [1, 1]]),
            )
            b1_sb = psingle.tile([R, 1], F32, name="b1_sb")
            nc.gpsimd.dma_start(
                out=b1_sb[:],
                in_=bass.AP(tensor=b1_d.ap().tensor, offset=0, ap=[[1, R], [1, 1]]),
            )
            b2_sb = psingle.tile([128, 2], F32, name="b2_sb")
            nc.gpsimd.dma_start(out=b2_sb[:], in_=b2_d[:].rearrange("(h c) -> c h", c=128))

            # w1T[c, h, r] = w1[r, h*128+c]
            w1_nat = psingle.tile([R, 2, 128], F32, name="w1_nat")
            nc.gpsimd.dma_start(out=w1_nat[:], in_=w1_d[:].rearrange("r (h c) -> r h c", c=128))
            w1T_ps = ppsX.tile([128, 2, R], F32, tag="psx", name="w1T_ps")
            for h in range(2):
                nc.tensor.transpose(w1T_ps[:, h, :], w1_nat[:, h, :], ident[0:R, 0:R])
            w1T = psingle.tile([128, 2, R], F32, name="w1T")
            nc.vector.tensor_copy(out=w1T[:], in_=w1T_ps[:])

            # w2T[r, h*128+c] = w2[h*128+c, r]
            w2_nat = psingle.tile([128, 2, R], F32, name="w2_nat")
            nc.gpsimd.dma_start(out=w2_nat[:], in_=w2_d[:].rearrange("(h c) r -> c h r", c=128))
            w2T = psingle.tile([R, 2, 128], F32, name="w2T")
            for h in range(2):
                w2T_ps = ppsX.tile([R, 128], F32, tag="psx", name=f"w2T_ps_{h}")
                nc.tensor.transpose(w2T_ps[:], w2_nat[:, h, :], ident[:])
                nc.vector.tensor_copy(out=w2T[:, h, :], in_=w2T_ps[:])

            # ---------------- per sample (software-pipelined) ----------------
            x16 = {}
            pp = {}
            psE = {}
            attT = {}
            rs = {}

            def emit_load(b):
                pp[b] = psmall.tile([128, 2, NSEG], F32, tag="pp", name=f"pp_{b}")
                x16[b] = [
                    px.tile([128, N], F16, tag="x16", name=f"x_{b}_{h}")
                    for h in range(2)
                ]
                for g in range(NSEG):
                    sl = slice(SEG * g, SEG * (g + 1))
                    for h in range(2):
                        st = pstage.tile([128, SEG], F32, tag="stage", name=f"st_{b}_{h}_{g}")
                        nc.sync.dma_start(
                            out=st[:], in_=x_d[b, 128 * h:128 * (h + 1), sl],
                        )
                        nc.scalar.activation(
                            out=x16[b][h][:, sl], in_=st[:],
                            func=mybir.ActivationFunctionType.Copy,
                            accum_out=pp[b][:, h, g:g + 1],
                        )

            def emit_phase1_group(b, gi):
                # chunks k = GRP*gi .. GRP*gi+GRP-1; one fp16 PSUM bank holds 4 xT chunks
                xT_ps = ppsX.tile([128, GRP, 256], F16, tag="psx", name=f"xTps_{b}_{gi}")
                for q in range(GRP):
                    k = GRP * gi + q
                    for h in range(2):
                        nc.tensor.transpose(
                            xT_ps[:, q, 128 * h:128 * (h + 1)],
                            x16[b][h][:, 128 * k:128 * (k + 1)],
                            ident16[:],
                        )
                xT = pxT.tile([128, GRP, 256], F16, tag="xT", name=f"xT_{b}_{gi}")
                nc.vector.tensor_copy(out=xT[:], in_=xT_ps[:])
                for q in range(GRP):
                    for h in range(2):
                        nc.tensor.matmul(
                            psE[b][:, 256 * h:256 * (h + 1)],
                            xT[:, q, 128 * h:128 * (h + 1)],
                            xT[:, q, :],
                            start=(gi == 0 and q == 0 and h == 0),
                            stop=(gi == NCH // GRP - 1 and q == GRP - 1 and h == 1),
                        )

            def alloc_psE(b):
                psE[b] = ppsE.tile([128, 512], F32, tag="psE", name=f"psE_{b}")

            def emit_phase1(b):
                alloc_psE(b)
                for gi in range(NCH // GRP):
                    emit_phase1_group(b, gi)

            def emit_epilogue(b):
                # SE gate (w1 pre-scaled by 1/N on host)
                pooled = psmall.tile([128, 2], F32, tag="pooled", name=f"pooled_{b}")
                for h in range(2):
                    nc.vector.reduce_sum(
                        out=pooled[:, h:h + 1], in_=pp[b][:, h, :], axis=mybir.AxisListType.X,
                    )
                hid_ps = ppsX.tile([R, 1], F32, tag="psx", name=f"hid_ps_{b}")
                for h in range(2):
                    nc.tensor.matmul(
                        hid_ps[:], w1T[:, h, :], pooled[:, h:h + 1],
                        start=(h == 0), stop=(h == 1),
                    )
                hid = psmall.tile([R, 1], F32, tag="hid", name=f"hid_{b}")
                nc.scalar.activation(
                    out=hid[:], in_=hid_ps[:],
                    func=mybir.ActivationFunctionType.Relu, bias=b1_sb[:], scale=1.0,
                )
                gg = psmall.tile([128, 2], F32, tag="gg", name=f"gg_{b}")
                for h in range(2):
                    gate_ps = ppsX.tile([128, 1], F32, tag="psx", name=f"gate_ps_{b}_{h}")
                    nc.tensor.matmul(gate_ps[:], w2T[:, h, :], hid[:])
                    nc.scalar.activation(
                        out=gg[:, h:h + 1], in_=gate_ps[:],
                        func=mybir.ActivationFunctionType.Sigmoid,
                        bias=b2_sb[:, h:h + 1], scale=1.0,
                    )
                nc.vector.tensor_scalar_mul(out=gg[:], in0=gg[:], scalar1=gamma_sb[:])

                # softmax rows: unnormalized fp16 att straight from ACT exp;
                # fold 1/sum * gamma * gate into per-row scale rs[b][h]
                att = []
                rs[b] = psmall.tile([128, 2], F32, tag="rs", name=f"rs_{b}")
                for h in range(2):
                    pE = psE[b][:, 256 * h:256 * (h + 1)]
                    mn = psmall.tile([128, 1], F32, tag="mn", name=f"mn_{b}_{h}")
                    nc.vector.tensor_reduce(
                        out=mn[:], in_=pE,
                        axis=mybir.AxisListType.X, op=mybir.AluOpType.min,
                    )
                    s = psmall.tile([128, 1], F32, tag="s", name=f"s_{b}_{h}")
                    at = patt.tile([128, 256], F16, tag=f"att{h}", name=f"att_{b}_{h}")
                    nc.scalar.activation(
                        out=at[:], in_=pE,
                        func=mybir.ActivationFunctionType.Exp,
                        bias=mn[:], scale=-1.0, accum_out=s[:],
                    )
                    srec = psmall.tile([128, 1], F32, tag="srec", name=f"srec_{b}_{h}")
                    nc.vector.reciprocal(out=srec[:], in_=s[:])
                    nc.vector.tensor_mul(
                        out=rs[b][:, h:h + 1], in0=srec[:], in1=gg[:, h:h + 1])
                    att.append(at)

                # transpose attention (fp16 PE transposes)
                attT[b] = patt.tile([128, 2, 256], F16, tag="attT", name=f"attT_{b}")
                for j in range(2):
                    attT_ps = ppsX.tile([128, 256], F16, tag="psx", name=f"attTps_{b}_{j}")
                    for h in range(2):
                        nc.tensor.transpose(
                            attT_ps[:, 128 * h:128 * (h + 1)],
                            att[h][:, 128 * j:128 * (j + 1)],
                            ident16[:],
                        )
                    nc.vector.tensor_copy(out=attT[b][:, j, :], in_=attT_ps[:])

            def emit_phase2_chunk(b, h, ci, col0, cw):
                # one staging chunk: cw columns of row-half h, then DMA out
                o_sb = pout.tile([128, OUTCH], F32, tag="osb", name=f"o_{b}_{ci}_{h}")
                nslab = cw // NT
                for t in range(nslab):
                    n0 = col0 + t * NT
                    pso = ppsO.tile([128, NT], F32, tag="ps_o", name=f"pso_{b}_{ci}_{t}_{h}")
                    for j in range(2):
                        nc.tensor.matmul(
                            pso[:],
                            attT[b][:, j, 128 * h:128 * (h + 1)],
                            x16[b][j][:, n0:n0 + NT],
                            start=(j == 0), stop=(j == 1),
                        )
                    # out = pso * (gamma*gate/sum) + x   (fused; GPSIMD can't
                    # read PSUM, so these all live on the DVE)
                    nc.vector.scalar_tensor_tensor(
                        out=o_sb[:, t * NT:t * NT + NT],
                        in0=pso[:],
                        scalar=rs[b][:, h:h + 1],
                        in1=x16[b][h][:, n0:n0 + NT],
                        op0=mybir.AluOpType.mult,
                        op1=mybir.AluOpType.add,
                    )
                nc.sync.dma_start(
                    out=out_d[b, 128 * h:128 * (h + 1), col0:col0 + cw],
                    in_=o_sb[:, 0:cw],
                )

            def phase2_chunks(b):
                ncols = [OUTCH] * (N // OUTCH) + ([N % OUTCH] if N % OUTCH else [])
                for h in range(2):
                    col0 = 0
                    for ci, cw in enumerate(ncols):
                        yield (b, h, ci, col0, cw)
                        col0 += cw

            emit_load(0)
            emit_load(1)
            emit_phase1(0)
            emit_epilogue(0)
            # interleave sample-0 phase 2 with sample-1 phase 1 so the PE can
            # fill load-DMA waits with ready matmul work
            p2q = list(phase2_chunks(0))
            alloc_psE(1)
            for gi in range(NCH // GRP):
                emit_phase1_group(1, gi)
                if gi % 2 == 0 and p2q:
                    emit_phase2_chunk(*p2q.pop(0))
            emit_epilogue(1)
            for args in p2q:
                emit_phase2_chunk(*args)
            for args in phase2_chunks(1):
                emit_phase2_chunk(*args)

    nc.finalize()
    return nc


_CACHE = {}


def get_nc():
    if "nc" not in _CACHE:
        _CACHE["nc"] = build_nc()
    return _CACHE["nc"]


def kernel_with_result(x, gamma, w1, b1, w2, b2, trace=False, **_ignored):
    x = np.asarray(x, dtype=np.float32)
    nc = get_nc()
    params = {
        "gamma": np.asarray(gamma, np.float32).reshape(1),
        "w1": np.asarray(w1, np.float32) * np.float32(1.0 / N),
        "b1": np.asarray(b1, np.float32),
        "w2": np.asarray(w2, np.float32),
        "b2": np.asarray(b2, np.float32),
        "ident": np.eye(128, dtype=np.float32),
    }
    xr = x.reshape(B, C, N)
    in_maps = [dict(params, x=xr[i * BL:(i + 1) * BL]) for i in range(NCORES)]
    res = run_bass_kernel_spmd(nc, in_maps, core_ids=list(range(NCORES)), trace=trace)
    out = np.concatenate([res.results[i]["out"] for i in range(NCORES)], axis=0)
    return out.reshape(B, C, H, W), res


def kernel(x, gamma, w1, b1, w2, b2, **_ignored):
    out, _res = kernel_with_result(x, gamma, w1, b1, w2, b2, trace=False)
    return out
